# revision 3
# baseline (speedup 1.0000x reference)
"""Trainium2 Bass kernel for nn_BinaryPooling2d (3x3 binary pooling -> per-(B,C) scalar).

Math (per (B,C) plane, per output pixel p with 3x3 taps t_k, center c):
  S1 = sum t_k ; S2 = sum t_k^2 ; mx = max t_k ; M = sum min(t_k, c)
  thr = mean|t_k - c| = (S1 + 9c - 2M)/9        [|a-b| = a+b-2min(a,b)]
  r   = c + thr = 2c + S1/9 - (2/9) M
  bv  = #{k: t_k >= r} ; m = S1/9 ; std = sqrt(S2/9 - m^2)
  out_pix = mx + (bv - m) * (std - mx) / 255 ;  out = mean_p out_pix

Layout per core: partition = (batch,channel) plane (2*64 = 128), free dim = (H,W).
All 3x3 taps are free-dim shifted views. Tap sums (S1, S2, M, bv) are computed
on the TensorEngine as identity-matmul accumulations into PSUM (exact fp32 sums
of bf16 values). Elementwise work runs on DVE in bf16 (2x mode); squares/sqrt
on ScalarE. Spatial sums ride for free on scalar_tensor_tensor accum_out.
Sharding: batch dim across 8 cores (pure data parallel).
"""

import sys

import numpy as np

if "/opt/trn_rl_repo" not in sys.path:
    sys.path.insert(0, "/opt/trn_rl_repo")

P = 128      # planes per core = partitions
H = W = 128
KS = 3
HO = WO = 126          # output spatial dims
BAND = 18              # output rows per band
NBANDS = HO // BAND    # 7
SUB = 3                # output rows per PE subtile
NSUB = BAND // SUB     # 6
NFREE = SUB * WO       # 378 (<= 512 psum bank)
NPIX = HO * WO
NACC = NBANDS * (1 + NSUB)   # accumulator slots (Smx per band + Sac per subtile)

TAPS = [(i, j) for i in range(KS) for j in range(KS)]

_CACHE = {}


def _split_multiwait_instructions(nc):
    """This walrus build rejects instructions with >1 sync wait. Hoist extra
    waits onto same-engine NoOps inserted before the instruction (sequential
    execution; sem conditions are monotonic, so semantics are identical)."""
    from concourse import mybir

    n = 0
    for f in nc.m.functions:
        for bb in f.blocks:
            out = []
            changed = False
            for ins in bb.instructions:
                si = ins.sync_info
                waits = list(si.on_wait) if si is not None else []
                if len(waits) > 1:
                    for k, w in enumerate(waits[:-1]):
                        out.append(mybir.InstNoOp(
                            name=f"{ins.name}-sw{k}",
                            sync_info=mybir.SyncInfo(on_wait=[w], on_update=[]),
                            bass_nofuse=True,
                            engine=ins.engine,
                        ))
                        n += 1
                    ins.sync_info = mybir.SyncInfo(
                        on_wait=[waits[-1]], on_update=list(si.on_update))
                    changed = True
                out.append(ins)
            if changed:
                bb.instructions = out
    return n


def _emit(nc, tile, mybir):
    f32 = mybir.dt.float32
    bf16 = mybir.dt.bfloat16
    A = mybir.AluOpType
    AF = mybir.ActivationFunctionType

    x_d = nc.dram_tensor("x", [P, H, W], f32, kind="ExternalInput")
    id_d = nc.dram_tensor("ident", [P, P], bf16, kind="ExternalInput")
    out_d = nc.dram_tensor("out", [P, 1], f32, kind="ExternalOutput")

    def fl(ap):
        return ap.rearrange("p a b -> p (a b)")

    with tile.TileContext(nc) as tc:
        with (
            tc.tile_pool(name="singles", bufs=1) as singles,
            tc.tile_pool(name="band", bufs=1) as band,
            tc.tile_pool(name="psA", bufs=2, space="PSUM") as psA,
            tc.tile_pool(name="psB", bufs=2, space="PSUM") as psB,
        ):
            xb = singles.tile([P, H, W], bf16)
            identT = singles.tile([P, P], bf16)
            accs = singles.tile([P, NACC], f32)
            tot = singles.tile([P, 1], f32)
            out_sb = singles.tile([P, 1], f32)

            nc.sync.dma_start(out=identT[:], in_=id_d[:])
            # HBM load with fp32->bf16 cast in the DMA (SWDGE)
            nc.gpsimd.dma_start(out=xb[:], in_=x_d[:])

            xbf = fl(xb[:])  # [P, H*W]
            n_acc = 0

            def acc_slot():
                nonlocal n_acc
                s = accs[:, n_acc:n_acc + 1]
                n_acc += 1
                return s

            for ib in range(NBANDS):
                y0 = ib * BAND
                NR = BAND + 2  # input rows this band

                xb1 = band.tile([P, NR, W], bf16)
                xxb = band.tile([P, NR, W], bf16)
                cb = band.tile([P, BAND, WO], bf16)
                mha = band.tile([P, NR, WO], bf16)
                mh = band.tile([P, NR, WO], bf16)
                mxa = band.tile([P, BAND, WO], bf16)
                mxb = band.tile([P, BAND, WO], bf16)
                mins = band.tile([P, 8, BAND, WO], bf16)
                isge = band.tile([P, 9, BAND, WO], bf16)
                mb = band.tile([P, BAND, WO], bf16)
                zb = band.tile([P, BAND, WO], bf16)
                rb = band.tile([P, BAND, WO], bf16)
                stdb = band.tile([P, BAND, WO], bf16)
                ub = band.tile([P, BAND, WO], bf16)
                junk = band.tile([P, NFREE], bf16)
                nmt = band.tile([P, NFREE], bf16)
                s1sq = band.tile([P, NFREE], f32)
                vart = band.tile([P, NFREE], f32)

                # xb1[p, t] = xb_band[p, t+1]  (flat shift for 4B-aligned odd taps)
                nflat = NR * W
                nc.vector.tensor_copy(
                    fl(xb1[:])[:, 0:nflat - 1],
                    xbf[:, y0 * W + 1: y0 * W + nflat],
                )
                # squares (incl halo rows) on ScalarE
                nc.scalar.activation(xxb[:], xb[:, y0:y0 + NR, :], AF.Square)
                # center plane, compact
                cv = xb1[:, 1:1 + BAND, 0:WO]
                nc.vector.tensor_copy(cb[:], cv)

                # horizontal 3-max over all input rows
                nc.vector.tensor_tensor(
                    mha[:], xb[:, y0:y0 + NR, 0:WO], xb1[:, :, 0:WO], A.max)
                nc.vector.tensor_tensor(
                    mh[:], mha[:], xb[:, y0:y0 + NR, 2:W], A.max)
                # vertical 3-max; second op also accumulates sum(mx) per partition
                nc.vector.tensor_tensor(
                    mxa[:], mh[:, 0:BAND, :], mh[:, 1:BAND + 1, :], A.max)
                nc.vector.scalar_tensor_tensor(
                    fl(mxb[:]), fl(mxa[:]), 1.0, fl(mh[:, 2:BAND + 2, :]),
                    A.mult, A.max, accum_out=acc_slot())

                # min(t_k, c) planes (8 taps, center excluded)
                kidx = 0
                for (i, j) in TAPS:
                    if (i, j) == (1, 1):
                        continue
                    if j == 1:
                        tv = xb1[:, i:i + BAND, 0:WO]
                    else:
                        tv = xb[:, y0 + i:y0 + i + BAND, j:j + WO]
                    nc.vector.tensor_tensor(mins[:, kidx], tv, cv, A.min)
                    kidx += 1

                # ---- subtile loop A: PE sums + stats -> m, z, r, std ----
                for s in range(NSUB):
                    r0 = s * SUB
                    s1ps = psA.tile([P, NFREE], f32, tag="s1ps")
                    s2ps = psA.tile([P, NFREE], f32, tag="s2ps")
                    mps = psA.tile([P, NFREE], f32, tag="mps")
                    for idx, (i, j) in enumerate(TAPS):
                        nc.tensor.matmul(
                            s1ps[:], identT[:],
                            xb[:, y0 + r0 + i: y0 + r0 + i + SUB, j:j + WO],
                            start=(idx == 0), stop=(idx == 8))
                    for idx, (i, j) in enumerate(TAPS):
                        nc.tensor.matmul(
                            s2ps[:], identT[:],
                            xxb[:, r0 + i: r0 + i + SUB, j:j + WO],
                            start=(idx == 0), stop=(idx == 8))
                    for k in range(8):
                        nc.tensor.matmul(
                            mps[:], identT[:], mins[:, k, r0:r0 + SUB, :],
                            start=(k == 0), stop=(k == 7))

                    msl = fl(mb[:, r0:r0 + SUB, :])
                    zsl = fl(zb[:, r0:r0 + SUB, :])
                    rsl = fl(rb[:, r0:r0 + SUB, :])
                    stdsl = fl(stdb[:, r0:r0 + SUB, :])
                    cbsl = fl(cb[:, r0:r0 + SUB, :])

                    # m = S1/9
                    nc.vector.tensor_scalar(msl, s1ps[:], 1.0 / 9.0, None, A.mult)
                    # z = 2c + m
                    nc.vector.scalar_tensor_tensor(
                        zsl, cbsl, 2.0, msl, A.mult, A.add)
                    # r = z - (2/9) M
                    nc.vector.scalar_tensor_tensor(
                        rsl, mps[:], -2.0 / 9.0, zsl, A.mult, A.add)
                    # s1sq = (1-1e-6) * S1^2 / 9  (epsilon keeps var >= 0)
                    nc.scalar.activation(
                        s1sq[:], s1ps[:], AF.Square,
                        scale=(1.0 / 3.0) * (1.0 - 5e-7))
                    nc.vector.tensor_tensor(vart[:], s2ps[:], s1sq[:], A.subtract)
                    # std = sqrt(var/9)
                    nc.scalar.activation(stdsl, vart[:], AF.Sqrt, scale=1.0 / 9.0)

                # ---- compare planes: [t_k >= r] ----
                for idx, (i, j) in enumerate(TAPS):
                    if j == 1:
                        tv = xb1[:, i:i + BAND, 0:WO]
                    else:
                        tv = xb[:, y0 + i:y0 + i + BAND, j:j + WO]
                    nc.vector.tensor_tensor(isge[:, idx], tv, rb[:], A.is_ge)

                # u' = std - mx
                nc.vector.tensor_tensor(ub[:], stdb[:], mxb[:], A.subtract)

                # ---- subtile loop B: bv + masked-sum accumulation ----
                for s in range(NSUB):
                    r0 = s * SUB
                    bvps = psB.tile([P, NFREE], f32, tag="bvps")
                    for idx in range(9):
                        nc.tensor.matmul(
                            bvps[:], identT[:], isge[:, idx, r0:r0 + SUB, :],
                            start=(idx == 0), stop=(idx == 8))
                    msl = fl(mb[:, r0:r0 + SUB, :])
                    usl = fl(ub[:, r0:r0 + SUB, :])
                    # nm = bv - m
                    nc.vector.tensor_tensor(nmt[:], bvps[:], msl, A.subtract)
                    # junk = (nm/255) * u' ; accum += sum(junk)
                    nc.vector.scalar_tensor_tensor(
                        junk[:], nmt[:], 1.0 / 255.0, usl, A.mult, A.mult,
                        accum_out=acc_slot())

            assert n_acc == NACC
            # total = sum of all accumulator slots, then / NPIX
            nc.vector.tensor_reduce(
                tot[:], accs[:], mybir.AxisListType.X, A.add)
            nc.vector.tensor_scalar(
                out_sb[:], tot[:], 1.0 / float(NPIX), None, A.mult)
            nc.sync.dma_start(out=out_d[:], in_=out_sb[:])

    n = _split_multiwait_instructions(nc)
    return nc


def _get_nc():
    if "nc" not in _CACHE:
        import concourse.bass as bass
        import concourse.tile as tile
        from concourse import mybir

        nc = bass.Bass()
        _emit(nc, tile, mybir)
        _CACHE["nc"] = nc
    return _CACHE["nc"]


def _run(x, trace=False, **kw):
    """x: (16,64,128,128) fp32. Returns (out (16,64,1,1) fp32, BassKernelResults)."""
    from concourse.bass_utils import run_bass_kernel_spmd
    import ml_dtypes

    nc = _get_nc()
    ident = np.eye(P, dtype=ml_dtypes.bfloat16)
    n_cores = 8
    per = x.shape[0] // n_cores
    in_maps = []
    for r in range(n_cores):
        shard = np.ascontiguousarray(
            x[r * per:(r + 1) * per], dtype=np.float32).reshape(P, H, W)
        in_maps.append({"x": shard, "ident": ident})
    res = run_bass_kernel_spmd(
        nc, in_maps, core_ids=list(range(n_cores)), trace=trace, **kw)
    outs = [res.results[r]["out"].reshape(per, 64, 1, 1) for r in range(n_cores)]
    return np.concatenate(outs, axis=0).astype(np.float32), res


def kernel(**inputs):
    out, _ = _run(np.asarray(inputs["x"]))
    return out


# revision 5
# speedup vs baseline: 1.0992x; 1.0992x over previous
"""Trainium2 Bass kernel for nn_BinaryPooling2d (3x3 binary pooling -> per-(B,C) scalar).

Math (per (B,C) plane, per output pixel p with 3x3 taps t_k, center c):
  S1 = sum t_k ; S2 = sum t_k^2 ; mx = max t_k ; M = sum min(t_k, c)
  thr = mean|t_k - c| = (S1 + 9c - 2M)/9        [|a-b| = a+b-2min(a,b)]
  r   = c + thr = 2c + S1/9 - (2/9) M
  bv  = #{k: t_k >= r} ; m = S1/9 ; std = sqrt(S2/9 - m^2)
  out_pix = mx + (bv - m) * (std - mx) / 255 ;  out = mean_p out_pix

Layout per core: partition = (batch,channel) plane (2*64 = 128), free dim = (H,W).
All 3x3 taps are free-dim shifted views. Tap sums (S1, S2, M, bv) run on the
TensorEngine as identity-matmul accumulations into PSUM (exact fp32 sums of
bf16 values). Elementwise work on DVE in bf16 (2x mode); squares/sqrt on
ScalarE. Spatial sums ride free on scalar_tensor_tensor accum_out. Input
arrives as 4 large fp32 HWDGE loads (fast path), cast to bf16 on-chip.
Sharding: batch dim across 8 cores (pure data parallel).
"""

import sys

import numpy as np

if "/opt/trn_rl_repo" not in sys.path:
    sys.path.insert(0, "/opt/trn_rl_repo")

P = 128      # planes per core = partitions
H = W = 128
KS = 3
HO = WO = 126          # output spatial dims
QROWS = 32             # output rows per quarter-load
BAND = 8               # output rows per band
SUB = 4                # output rows per PE subtile (NFREE = 504 <= 512)
NPIX = HO * WO

TAPS = [(i, j) for i in range(KS) for j in range(KS)]

_CACHE = {}


def _patch_run_command():
    """(ldw-opt dedup is broken in this walrus build — keep defaults.)"""
    pass


def _split_multiwait_instructions(nc):
    """This walrus build rejects instructions with >1 sync wait. Hoist extra
    waits onto same-engine NoOps inserted before the instruction (sequential
    execution; sem conditions are monotonic, so semantics are identical)."""
    from concourse import mybir

    n = 0
    for f in nc.m.functions:
        for bb in f.blocks:
            out = []
            changed = False
            for ins in bb.instructions:
                si = ins.sync_info
                waits = list(si.on_wait) if si is not None else []
                if len(waits) > 1:
                    for k, w in enumerate(waits[:-1]):
                        out.append(mybir.InstNoOp(
                            name=f"{ins.name}-sw{k}",
                            sync_info=mybir.SyncInfo(on_wait=[w], on_update=[]),
                            bass_nofuse=True,
                            engine=ins.engine,
                        ))
                        n += 1
                    ins.sync_info = mybir.SyncInfo(
                        on_wait=[waits[-1]], on_update=list(si.on_update))
                    changed = True
                out.append(ins)
            if changed:
                bb.instructions = out
    return n


def _bands():
    """Yield (quarter, in_rows_of_quarter, band_local_row0, band_out_rows)."""
    for q in range(4):
        qrows = 34 if q < 3 else 32           # input rows loaded
        qout = QROWS if q < 3 else HO - 3 * QROWS   # 32,32,32,30
        y = 0
        while y < qout:
            b = min(BAND, qout - y)
            yield q, qrows, y, b
            y += b


def _emit(nc, tile, mybir):
    f32 = mybir.dt.float32
    bf16 = mybir.dt.bfloat16
    A = mybir.AluOpType
    AF = mybir.ActivationFunctionType

    x_d = nc.dram_tensor("x", [P, H, W], f32, kind="ExternalInput")
    id_d = nc.dram_tensor("ident", [P, P], bf16, kind="ExternalInput")
    out_d = nc.dram_tensor("out", [P, 1], f32, kind="ExternalOutput")

    def fl(ap):
        return ap.rearrange("p a b -> p (a b)")

    nacc_total = sum(1 + (bo + SUB - 1) // SUB for _, _, _, bo in _bands())

    with tile.TileContext(nc) as tc:
        with (
            tc.tile_pool(name="singles", bufs=1) as singles,
            tc.tile_pool(name="quarters", bufs=2) as quarters,
            tc.tile_pool(name="band", bufs=2) as band,
            tc.tile_pool(name="psA", bufs=2, space="PSUM") as psA,
            tc.tile_pool(name="psB", bufs=2, space="PSUM") as psB,
        ):
            identT = singles.tile([P, P], bf16)
            accs = singles.tile([P, nacc_total], f32)
            tot = singles.tile([P, 1], f32)
            out_sb = singles.tile([P, 1], f32)

            nc.sync.dma_start(out=identT[:], in_=id_d[:])

            n_acc = 0

            def acc_slot():
                nonlocal n_acc
                s = accs[:, n_acc:n_acc + 1]
                n_acc += 1
                return s

            cur_q = -1
            xq = None
            for q, qrows, yl, BO in _bands():
                if q != cur_q:
                    cur_q = q
                    xq = quarters.tile([P, 34, W], f32, tag="xq")
                    nc.sync.dma_start(
                        out=xq[:, 0:qrows, :],
                        in_=x_d[:, q * QROWS: q * QROWS + qrows, :])

                NR = BO + 2
                NSUBS = [SUB] * (BO // SUB) + ([BO % SUB] if BO % SUB else [])

                xbb = band.tile([P, BAND + 2, W], bf16)
                xb1 = band.tile([P, BAND + 2, W], bf16)
                xxb = band.tile([P, BAND + 2, W], bf16)
                cb = band.tile([P, BAND, WO], bf16)
                mha = band.tile([P, BAND + 2, WO], bf16)
                mh = band.tile([P, BAND + 2, WO], bf16)
                mxa = band.tile([P, BAND, WO], bf16)
                mxb = band.tile([P, BAND, WO], bf16)
                mins = band.tile([P, 8, BAND, WO], bf16)
                isge = band.tile([P, 9, BAND, WO], bf16)
                mb = band.tile([P, BAND, WO], bf16)
                zb = band.tile([P, BAND, WO], bf16)
                rb = band.tile([P, BAND, WO], bf16)
                stdb = band.tile([P, BAND, WO], bf16)
                ub = band.tile([P, BAND, WO], bf16)
                junk = band.tile([P, SUB * WO], bf16)
                nmt = band.tile([P, SUB * WO], bf16)
                s1sq = band.tile([P, SUB * WO], f32)
                vart = band.tile([P, SUB * WO], f32)

                xqf = fl(xq[:])
                # on-chip casts (RTN), including the +1-element shifted copy
                # (fp32 source keeps 4B alignment for the odd-column taps)
                nflat = NR * W
                nc.vector.tensor_copy(
                    fl(xbb[:])[:, 0:nflat], xqf[:, yl * W: yl * W + nflat])
                nc.vector.tensor_copy(
                    fl(xb1[:])[:, 0:nflat - 1],
                    xqf[:, yl * W + 1: yl * W + nflat])
                # squares (incl halo rows) on ScalarE
                nc.scalar.activation(
                    xxb[:, 0:NR, :], xbb[:, 0:NR, :], AF.Square)
                # center plane, compact
                cv = xb1[:, 1:1 + BO, 0:WO]
                nc.vector.tensor_copy(cb[:, 0:BO, :], cv)

                # horizontal 3-max over all input rows
                nc.vector.tensor_tensor(
                    mha[:, 0:NR, :], xbb[:, 0:NR, 0:WO], xb1[:, 0:NR, 0:WO],
                    A.max)
                nc.vector.tensor_tensor(
                    mh[:, 0:NR, :], mha[:, 0:NR, :], xbb[:, 0:NR, 2:W], A.max)
                # vertical 3-max; also accumulates sum(mx) per partition
                nc.vector.tensor_tensor(
                    mxa[:, 0:BO, :], mh[:, 0:BO, :], mh[:, 1:BO + 1, :], A.max)
                nc.vector.scalar_tensor_tensor(
                    fl(mxb[:, 0:BO, :]), fl(mxa[:, 0:BO, :]), 1.0,
                    fl(mh[:, 2:BO + 2, :]), A.mult, A.max,
                    accum_out=acc_slot())

                # min(t_k, c) planes (8 taps, center excluded)
                kidx = 0
                for (i, j) in TAPS:
                    if (i, j) == (1, 1):
                        continue
                    if j == 1:
                        tv = xb1[:, i:i + BO, 0:WO]
                    else:
                        tv = xbb[:, i:i + BO, j:j + WO]
                    nc.vector.tensor_tensor(mins[:, kidx, 0:BO, :], tv, cv, A.min)
                    kidx += 1

                # ---- subtile loop A: PE sums + stats -> m, z, r, std ----
                r0 = 0
                for sb in NSUBS:
                    nf = sb * WO
                    s1ps = psA.tile([P, SUB * WO], f32, tag="s1ps")
                    s2ps = psA.tile([P, SUB * WO], f32, tag="s2ps")
                    mps = psA.tile([P, SUB * WO], f32, tag="mps")
                    for idx, (i, j) in enumerate(TAPS):
                        nc.tensor.matmul(
                            s1ps[:, 0:nf], identT[:],
                            xbb[:, r0 + i: r0 + i + sb, j:j + WO],
                            start=(idx == 0), stop=(idx == 8))
                    for idx, (i, j) in enumerate(TAPS):
                        nc.tensor.matmul(
                            s2ps[:, 0:nf], identT[:],
                            xxb[:, r0 + i: r0 + i + sb, j:j + WO],
                            start=(idx == 0), stop=(idx == 8))
                    for k in range(8):
                        nc.tensor.matmul(
                            mps[:, 0:nf], identT[:],
                            mins[:, k, r0:r0 + sb, :],
                            start=(k == 0), stop=(k == 7))

                    msl = fl(mb[:, r0:r0 + sb, :])
                    zsl = fl(zb[:, r0:r0 + sb, :])
                    rsl = fl(rb[:, r0:r0 + sb, :])
                    stdsl = fl(stdb[:, r0:r0 + sb, :])
                    cbsl = fl(cb[:, r0:r0 + sb, :])

                    # m = S1/9
                    nc.vector.tensor_scalar(
                        msl, s1ps[:, 0:nf], 1.0 / 9.0, None, A.mult)
                    # z = 2c + m
                    nc.vector.scalar_tensor_tensor(
                        zsl, cbsl, 2.0, msl, A.mult, A.add)
                    # r = z - (2/9) M
                    nc.vector.scalar_tensor_tensor(
                        rsl, mps[:, 0:nf], -2.0 / 9.0, zsl, A.mult, A.add)
                    # s1sq = (1-1e-6) * S1^2 / 9  (epsilon keeps var >= 0)
                    nc.scalar.activation(
                        s1sq[:, 0:nf], s1ps[:, 0:nf], AF.Square,
                        scale=(1.0 / 3.0) * (1.0 - 5e-7))
                    nc.vector.tensor_tensor(
                        vart[:, 0:nf], s2ps[:, 0:nf], s1sq[:, 0:nf], A.subtract)
                    # std = sqrt(var/9)
                    nc.scalar.activation(
                        stdsl, vart[:, 0:nf], AF.Sqrt, scale=1.0 / 9.0)
                    r0 += sb

                # ---- compare planes: [t_k >= r] ----
                for idx, (i, j) in enumerate(TAPS):
                    if j == 1:
                        tv = xb1[:, i:i + BO, 0:WO]
                    else:
                        tv = xbb[:, i:i + BO, j:j + WO]
                    nc.vector.tensor_tensor(
                        isge[:, idx, 0:BO, :], tv, rb[:, 0:BO, :], A.is_ge)

                # u' = std - mx
                nc.vector.tensor_tensor(
                    ub[:, 0:BO, :], stdb[:, 0:BO, :], mxb[:, 0:BO, :],
                    A.subtract)

                # ---- subtile loop B: bv + masked-sum accumulation ----
                r0 = 0
                for sb in NSUBS:
                    nf = sb * WO
                    bvps = psB.tile([P, SUB * WO], f32, tag="bvps")
                    for idx in range(9):
                        nc.tensor.matmul(
                            bvps[:, 0:nf], identT[:],
                            isge[:, idx, r0:r0 + sb, :],
                            start=(idx == 0), stop=(idx == 8))
                    msl = fl(mb[:, r0:r0 + sb, :])
                    usl = fl(ub[:, r0:r0 + sb, :])
                    # nm = bv - m
                    nc.vector.tensor_tensor(
                        nmt[:, 0:nf], bvps[:, 0:nf], msl, A.subtract)
                    # junk = (nm/255) * u' ; accum += sum(junk)
                    nc.vector.scalar_tensor_tensor(
                        junk[:, 0:nf], nmt[:, 0:nf], 1.0 / 255.0, usl,
                        A.mult, A.mult, accum_out=acc_slot())
                    r0 += sb

            assert n_acc == nacc_total, (n_acc, nacc_total)
            # total = sum of all accumulator slots, then / NPIX
            nc.vector.tensor_reduce(
                tot[:], accs[:], mybir.AxisListType.X, A.add)
            nc.vector.tensor_scalar(
                out_sb[:], tot[:], 1.0 / float(NPIX), None, A.mult)
            nc.sync.dma_start(out=out_d[:], in_=out_sb[:])

    _split_multiwait_instructions(nc)
    return nc


def _get_nc():
    if "nc" not in _CACHE:
        import concourse.bass as bass
        import concourse.tile as tile
        from concourse import mybir

        _patch_run_command()
        nc = bass.Bass()
        _emit(nc, tile, mybir)
        _CACHE["nc"] = nc
    return _CACHE["nc"]


def _run(x, trace=False, **kw):
    """x: (16,64,128,128) fp32. Returns (out (16,64,1,1) fp32, BassKernelResults)."""
    from concourse.bass_utils import run_bass_kernel_spmd
    import ml_dtypes

    nc = _get_nc()
    ident = np.eye(P, dtype=ml_dtypes.bfloat16)
    n_cores = 8
    per = x.shape[0] // n_cores
    in_maps = []
    for r in range(n_cores):
        shard = np.ascontiguousarray(
            x[r * per:(r + 1) * per], dtype=np.float32).reshape(P, H, W)
        in_maps.append({"x": shard, "ident": ident})
    res = run_bass_kernel_spmd(
        nc, in_maps, core_ids=list(range(n_cores)), trace=trace, **kw)
    outs = [res.results[r]["out"].reshape(per, 64, 1, 1) for r in range(n_cores)]
    return np.concatenate(outs, axis=0).astype(np.float32), res


def kernel(**inputs):
    out, _ = _run(np.asarray(inputs["x"]))
    return out


# revision 6
# speedup vs baseline: 1.1465x; 1.0430x over previous
"""Trainium2 Bass kernel for nn_BinaryPooling2d (3x3 binary pooling -> per-(B,C) scalar).

Math (per (B,C) plane, per output pixel p with 3x3 taps t_k, center c):
  S1 = sum t_k ; S2 = sum t_k^2 ; mx = max t_k ; M = sum min(t_k, c)
  thr = mean|t_k - c| = (S1 + 9c - 2M)/9        [|a-b| = a+b-2min(a,b)]
  r   = c + thr = 2c + S1/9 - (2/9) M
  bv  = #{k: t_k >= r} ; m = S1/9 ; std = sqrt(S2/9 - m^2)
  out_pix = mx + (bv - m) * (std - mx) / 255 ;  out = mean_p out_pix

Layout per core: partition = (batch,channel) plane (2*64 = 128), free dim =
(H,W). All 3x3 taps are free-dim shifted views. Tap sums (S1, S2, M, bv) run
on the TensorEngine as identity-matmul accumulations into PSUM (exact fp32
sums of bf16 values); the identity is loaded once (ldweights=False on repeat
matmuls). Elementwise work on DVE in bf16 (2x mode); squares/sqrt on ScalarE.
Spatial sums ride free on scalar_tensor_tensor accum_out. Input arrives as 4
large fp32 HWDGE loads, cast to bf16 on-chip. Bands are software-pipelined
(prep(b+1) before main(b), subtile-B deferred one band) so the PE stays fed.
Sharding: batch dim across 8 cores (pure data parallel).
"""

import sys

import numpy as np

if "/opt/trn_rl_repo" not in sys.path:
    sys.path.insert(0, "/opt/trn_rl_repo")

P = 128      # planes per core = partitions
H = W = 128
KS = 3
HO = WO = 126          # output spatial dims
QROWS = 32             # output rows per quarter-load
BAND = 8               # output rows per band
SUB = 4                # output rows per PE subtile (NFREE = 504 <= 512)
NPIX = HO * WO

TAPS = [(i, j) for i in range(KS) for j in range(KS)]

_CACHE = {}


def _split_multiwait_instructions(nc):
    """This walrus build rejects instructions with >1 sync wait. Hoist extra
    waits onto same-engine NoOps inserted before the instruction (sequential
    execution; sem conditions are monotonic, so semantics are identical)."""
    from concourse import mybir

    n = 0
    for f in nc.m.functions:
        for bb in f.blocks:
            out = []
            changed = False
            for ins in bb.instructions:
                si = ins.sync_info
                waits = list(si.on_wait) if si is not None else []
                if len(waits) > 1:
                    for k, w in enumerate(waits[:-1]):
                        out.append(mybir.InstNoOp(
                            name=f"{ins.name}-sw{k}",
                            sync_info=mybir.SyncInfo(on_wait=[w], on_update=[]),
                            bass_nofuse=True,
                            engine=ins.engine,
                        ))
                        n += 1
                    ins.sync_info = mybir.SyncInfo(
                        on_wait=[waits[-1]], on_update=list(si.on_update))
                    changed = True
                out.append(ins)
            if changed:
                bb.instructions = out
    return n


def _force_single_ldweights(nc):
    """All matmuls share the same stationary identity; keep the weight load
    only on the first one (walrus's ldw dedup pass is broken here)."""
    first = True
    n = 0
    for f in nc.m.functions:
        for bb in f.blocks:
            for ins in bb.instructions:
                if type(ins).__name__ == "InstMatmult":
                    if first:
                        first = False
                    else:
                        ins.ldweights = False
                        n += 1
    return n


def _bands():
    """Yield (quarter, in_rows_of_quarter, band_local_row0, band_out_rows)."""
    for q in range(4):
        qrows = 34 if q < 3 else 32
        qout = QROWS if q < 3 else HO - 3 * QROWS   # 32,32,32,30
        y = 0
        while y < qout:
            b = min(BAND, qout - y)
            yield q, qrows, y, b
            y += b


def _emit(nc, tile, mybir):
    f32 = mybir.dt.float32
    bf16 = mybir.dt.bfloat16
    A = mybir.AluOpType
    AF = mybir.ActivationFunctionType

    x_d = nc.dram_tensor("x", [P, H, W], f32, kind="ExternalInput")
    id_d = nc.dram_tensor("ident", [P, P], bf16, kind="ExternalInput")
    out_d = nc.dram_tensor("out", [P, 1], f32, kind="ExternalOutput")

    def fl(ap):
        return ap.rearrange("p a b -> p (a b)")

    bands = list(_bands())
    nacc_total = sum(1 + (bo + SUB - 1) // SUB for _, _, _, bo in bands)

    with tile.TileContext(nc) as tc:
        with (
            tc.tile_pool(name="singles", bufs=1) as singles,
            tc.tile_pool(name="quarters", bufs=2) as quarters,
            tc.tile_pool(name="band", bufs=2) as band,
            tc.tile_pool(name="psA", bufs=2, space="PSUM") as psA,
            tc.tile_pool(name="psB", bufs=2, space="PSUM") as psB,
        ):
            identT = singles.tile([P, P], bf16)
            accs = singles.tile([P, nacc_total], f32)
            tot = singles.tile([P, 1], f32)
            out_sb = singles.tile([P, 1], f32)

            nc.sync.dma_start(out=identT[:], in_=id_d[:])

            n_acc = 0

            def acc_slot():
                nonlocal n_acc
                s = accs[:, n_acc:n_acc + 1]
                n_acc += 1
                return s

            cur_q = [-1]
            xq_tile = [None]
            state = {}   # band index -> dict of tiles/views

            def prep(bi):
                q, qrows, yl, BO = bands[bi]
                if q != cur_q[0]:
                    cur_q[0] = q
                    xq = quarters.tile([P, 34, W], f32, tag="xq", name="xq")
                    nc.sync.dma_start(
                        out=xq[:, 0:qrows, :],
                        in_=x_d[:, q * QROWS: q * QROWS + qrows, :])
                    xq_tile[0] = xq
                xq = xq_tile[0]
                NR = BO + 2

                st = {}
                st["BO"] = BO
                xbb = band.tile([P, BAND + 2, W], bf16, name="xbb", tag="xbb")
                xb1 = band.tile([P, BAND + 2, W], bf16, name="xb1", tag="xb1")
                xxb = band.tile([P, BAND + 2, W], bf16, name="xxb", tag="xxb")
                cb = band.tile([P, BAND, WO], bf16, name="cb", tag="cb")
                mha = band.tile([P, BAND + 2, WO], bf16, name="mha", tag="mha")
                mh = band.tile([P, BAND + 2, WO], bf16, name="mh", tag="mh")
                mxa = band.tile([P, BAND, WO], bf16, name="mxa", tag="mxa")
                mxb = band.tile([P, BAND, WO], bf16, name="mxb", tag="mxb")
                mins = band.tile([P, 8, BAND, WO], bf16, name="mins", tag="mins")
                st.update(xbb=xbb, xb1=xb1, xxb=xxb, cb=cb, mxb=mxb, mins=mins)

                xqf = fl(xq[:])
                nflat = NR * W
                nc.vector.tensor_copy(
                    fl(xbb[:])[:, 0:nflat], xqf[:, yl * W: yl * W + nflat])
                nc.vector.tensor_copy(
                    fl(xb1[:])[:, 0:nflat - 1],
                    xqf[:, yl * W + 1: yl * W + nflat])
                nc.scalar.activation(
                    xxb[:, 0:NR, :], xbb[:, 0:NR, :], AF.Square)
                cv = xb1[:, 1:1 + BO, 0:WO]
                nc.vector.tensor_copy(cb[:, 0:BO, :], cv)

                nc.vector.tensor_tensor(
                    mha[:, 0:NR, :], xbb[:, 0:NR, 0:WO], xb1[:, 0:NR, 0:WO],
                    A.max)
                nc.vector.tensor_tensor(
                    mh[:, 0:NR, :], mha[:, 0:NR, :], xbb[:, 0:NR, 2:W], A.max)
                nc.vector.tensor_tensor(
                    mxa[:, 0:BO, :], mh[:, 0:BO, :], mh[:, 1:BO + 1, :], A.max)
                nc.vector.scalar_tensor_tensor(
                    fl(mxb[:, 0:BO, :]), fl(mxa[:, 0:BO, :]), 1.0,
                    fl(mh[:, 2:BO + 2, :]), A.mult, A.max,
                    accum_out=acc_slot())

                kidx = 0
                for (i, j) in TAPS:
                    if (i, j) == (1, 1):
                        continue
                    if j == 1:
                        tv = xb1[:, i:i + BO, 0:WO]
                    else:
                        tv = xbb[:, i:i + BO, j:j + WO]
                    nc.vector.tensor_tensor(
                        mins[:, kidx, 0:BO, :], tv, cv, A.min)
                    kidx += 1
                state[bi] = st

            def main_a(bi):
                st = state[bi]
                BO = st["BO"]
                xbb, xb1, xxb, cb, mxb, mins = (
                    st["xbb"], st["xb1"], st["xxb"], st["cb"], st["mxb"],
                    st["mins"])
                NSUBS = [SUB] * (BO // SUB) + ([BO % SUB] if BO % SUB else [])

                mb = band.tile([P, BAND, WO], bf16, name="mb", tag="mb")
                zb = band.tile([P, BAND, WO], bf16, name="zb", tag="zb")
                rb = band.tile([P, BAND, WO], bf16, name="rb", tag="rb")
                stdb = band.tile([P, BAND, WO], bf16, name="stdb", tag="stdb")
                ub = band.tile([P, BAND, WO], bf16, name="ub", tag="ub")
                isge = band.tile([P, 9, BAND, WO], bf16, name="isge", tag="isge")
                s1sq = band.tile([P, SUB * WO], f32, name="s1sq", tag="s1sq")
                vart = band.tile([P, SUB * WO], f32, name="vart", tag="vart")
                st.update(mb=mb, ub=ub, isge=isge)

                r0 = 0
                for sb in NSUBS:
                    nf = sb * WO
                    s1ps = psA.tile([P, SUB * WO], f32, tag="s1ps", name="s1ps")
                    s2ps = psA.tile([P, SUB * WO], f32, tag="s2ps", name="s2ps")
                    mps = psA.tile([P, SUB * WO], f32, tag="mps", name="mps")
                    for idx, (i, j) in enumerate(TAPS):
                        nc.tensor.matmul(
                            s1ps[:, 0:nf], identT[:],
                            xbb[:, r0 + i: r0 + i + sb, j:j + WO],
                            start=(idx == 0), stop=(idx == 8))
                    for idx, (i, j) in enumerate(TAPS):
                        nc.tensor.matmul(
                            s2ps[:, 0:nf], identT[:],
                            xxb[:, r0 + i: r0 + i + sb, j:j + WO],
                            start=(idx == 0), stop=(idx == 8))
                    for k in range(8):
                        nc.tensor.matmul(
                            mps[:, 0:nf], identT[:],
                            mins[:, k, r0:r0 + sb, :],
                            start=(k == 0), stop=(k == 7))

                    msl = fl(mb[:, r0:r0 + sb, :])
                    zsl = fl(zb[:, r0:r0 + sb, :])
                    rsl = fl(rb[:, r0:r0 + sb, :])
                    stdsl = fl(stdb[:, r0:r0 + sb, :])
                    cbsl = fl(cb[:, r0:r0 + sb, :])

                    nc.vector.tensor_scalar(
                        msl, s1ps[:, 0:nf], 1.0 / 9.0, None, A.mult)
                    nc.vector.scalar_tensor_tensor(
                        zsl, cbsl, 2.0, msl, A.mult, A.add)
                    nc.vector.scalar_tensor_tensor(
                        rsl, mps[:, 0:nf], -2.0 / 9.0, zsl, A.mult, A.add)
                    nc.scalar.activation(
                        s1sq[:, 0:nf], s1ps[:, 0:nf], AF.Square,
                        scale=(1.0 / 3.0) * (1.0 - 5e-7))
                    nc.vector.tensor_tensor(
                        vart[:, 0:nf], s2ps[:, 0:nf], s1sq[:, 0:nf],
                        A.subtract)
                    nc.scalar.activation(
                        stdsl, vart[:, 0:nf], AF.Sqrt, scale=1.0 / 9.0)
                    r0 += sb

                for idx, (i, j) in enumerate(TAPS):
                    if j == 1:
                        tv = xb1[:, i:i + BO, 0:WO]
                    else:
                        tv = xbb[:, i:i + BO, j:j + WO]
                    nc.vector.tensor_tensor(
                        isge[:, idx, 0:BO, :], tv, rb[:, 0:BO, :], A.is_ge)

                nc.vector.tensor_tensor(
                    ub[:, 0:BO, :], stdb[:, 0:BO, :], mxb[:, 0:BO, :],
                    A.subtract)

            def main_b(bi):
                st = state.pop(bi)
                BO = st["BO"]
                mb, ub, isge = st["mb"], st["ub"], st["isge"]
                NSUBS = [SUB] * (BO // SUB) + ([BO % SUB] if BO % SUB else [])
                nmt = band.tile([P, SUB * WO], bf16, name="nmt", tag="nmt")
                junk = band.tile([P, SUB * WO], bf16, name="junk", tag="junk")
                r0 = 0
                for sb in NSUBS:
                    nf = sb * WO
                    bvps = psB.tile([P, SUB * WO], f32, tag="bvps", name="bvps")
                    for idx in range(9):
                        nc.tensor.matmul(
                            bvps[:, 0:nf], identT[:],
                            isge[:, idx, r0:r0 + sb, :],
                            start=(idx == 0), stop=(idx == 8))
                    msl = fl(mb[:, r0:r0 + sb, :])
                    usl = fl(ub[:, r0:r0 + sb, :])
                    nc.vector.tensor_tensor(
                        nmt[:, 0:nf], bvps[:, 0:nf], msl, A.subtract)
                    nc.vector.scalar_tensor_tensor(
                        junk[:, 0:nf], nmt[:, 0:nf], 1.0 / 255.0, usl,
                        A.mult, A.mult, accum_out=acc_slot())
                    r0 += sb

            # software pipeline: prep one band ahead; defer B one band back
            prep(0)
            for bi in range(len(bands)):
                if bi + 1 < len(bands):
                    prep(bi + 1)
                main_a(bi)
                if bi > 0:
                    main_b(bi - 1)
            main_b(len(bands) - 1)

            assert n_acc == nacc_total, (n_acc, nacc_total)
            nc.vector.tensor_reduce(
                tot[:], accs[:], mybir.AxisListType.X, A.add)
            nc.vector.tensor_scalar(
                out_sb[:], tot[:], 1.0 / float(NPIX), None, A.mult)
            nc.sync.dma_start(out=out_d[:], in_=out_sb[:])

    _split_multiwait_instructions(nc)
    _force_single_ldweights(nc)
    return nc


def _get_nc():
    if "nc" not in _CACHE:
        import concourse.bass as bass
        import concourse.tile as tile
        from concourse import mybir

        nc = bass.Bass()
        _emit(nc, tile, mybir)
        _CACHE["nc"] = nc
    return _CACHE["nc"]


def _run(x, trace=False, **kw):
    """x: (16,64,128,128) fp32. Returns (out (16,64,1,1) fp32, BassKernelResults)."""
    from concourse.bass_utils import run_bass_kernel_spmd
    import ml_dtypes

    nc = _get_nc()
    ident = np.eye(P, dtype=ml_dtypes.bfloat16)
    n_cores = 8
    per = x.shape[0] // n_cores
    in_maps = []
    for r in range(n_cores):
        shard = np.ascontiguousarray(
            x[r * per:(r + 1) * per], dtype=np.float32).reshape(P, H, W)
        in_maps.append({"x": shard, "ident": ident})
    res = run_bass_kernel_spmd(
        nc, in_maps, core_ids=list(range(n_cores)), trace=trace, **kw)
    outs = [res.results[r]["out"].reshape(per, 64, 1, 1) for r in range(n_cores)]
    return np.concatenate(outs, axis=0).astype(np.float32), res


def kernel(**inputs):
    out, _ = _run(np.asarray(inputs["x"]))
    return out
